# revision 40
# baseline (speedup 1.0000x reference)
"""Causal cross-attention kernel for 8 TRN2 NeuronCores.

Sharding: data-parallel over batch (B=2) x tensor-parallel over head
groups (16 heads -> 4 groups of 4). Core c handles batch c//4, heads
[4*(c%4), 4*(c%4)+4). Each core computes its partial output projection
(w_out rows for its heads); the host sums the 4 partials per batch
(the "all-reduce"), adds b_out, and fixes the fully-masked row 0.

The kernel is PE-issue-limited under the HAM power throttle (sustained
PE duty is capped at ~50% after a ~65us full-rate grace window), so the
structure maximizes overlap: a single fused pipeline over s-chunks
  proj(sc) -> attention(sc, all heads) -> out-proj(sc)
with DMAs issued k-interleaved so the first matmul starts ~5us in, and
score/AV issue interleaved so the PE rides through the mask->exp
round-trip latency.

Device dataflow per core (all matmuls in natural layouts, no device
transposes; activations host-transposed once):
  qT[f',s] = sum_f wq[f,f'] xT_from[f,s]     (lhsT=wq tile, rhs=xT tile)
  kT[f',s] = sum_f wk[f,f'] xT_to[f,s]
  v[z,f']  = sum_f xT_to[f,z] wv[f,f']       (lhsT=xT tile, rhs=wv)
  scoresT[z,s] = sum_d kT[d,z] qT[d,s]       (per head, K=64)
  P = exp(scoresT + causal_mask)             (no max-subtraction; logits
                                              are bounded ~|50| so exp is
                                              safe in f32, masked -> 0)
  out'T[d|1,s] = sum_z v'[z,d|1] P[z,s]      (v' has a ones column ->
                                              row 64 = softmax denom)
  attn_outT = out'T[0:64] * (1/out'T[64])    (PE-broadcast of recip row)
  out[s,fo] = sum_hd attn_outT[hd,s] wo[hd,fo]
"""

import numpy as np
import ml_dtypes
import concourse.bass as bass
import concourse.mybir as mybir
import concourse.tile as tile
from concourse.bass_utils import run_bass_kernel_spmd

B, S, F, H = 2, 2048, 1024, 16
NCORES = 8
HG = 4          # head groups (tensor-parallel degree per batch)
HPC = H // HG   # heads per core = 4
D = F // H      # head dim = 64
CW = HPC * D    # per-core projection width = 256
MASK_VAL = 1.0e12
SC = 512        # s-chunk for projections / scores
NZC = S // 128  # 16 z-chunks

f32 = mybir.dt.float32
f32r = mybir.dt.float32r
bf16 = mybir.dt.bfloat16
fp16 = mybir.dt.float16

# Walrus encodes at most 1 sync wait on most TRN2 instructions; Tile can
# attach several. Redistribute excess waits onto preceding same-engine NOPs.


def _split_excess_waits(nc):
    for fn in nc.m.functions:
        for bb in fn.blocks:
            insts = list(bb.instructions)
            out = []
            changed = False
            for inst in insts:
                si = inst.sync_info
                waits = list(si.on_wait) if si is not None else []
                if len(waits) > 1:
                    changed = True
                    inst.sync_info = mybir.SyncInfo(
                        on_update=list(si.on_update), on_wait=waits[-1:]
                    )
                    for idx, w in enumerate(waits[:-1]):
                        nop = mybir.InstNoOp(name=f"{inst.name}-wsplit{idx}")
                        nop.engine = inst.engine
                        nop.sync_info = mybir.SyncInfo(on_update=[], on_wait=[w])
                        out.append(nop)
                out.append(inst)
            if changed:
                bb.instructions = out


def _round_f32r(x):
    u = np.ascontiguousarray(x, dtype=np.float32).view(np.uint32)
    u = ((u.astype(np.uint64) + 0x1000) & 0xFFFFE000).astype(np.uint32)
    return u.view(np.float32)


def _build():
    nc = bass.Bass()
    xf_d = nc.declare_dram_parameter("xf", [F, S], fp16, isOutput=False)
    xt_d = nc.declare_dram_parameter("xt", [F, S], fp16, isOutput=False)
    wq_d = nc.declare_dram_parameter("wq", [F, CW], fp16, isOutput=False)
    wk_d = nc.declare_dram_parameter("wk", [F, CW], fp16, isOutput=False)
    wv_d = nc.declare_dram_parameter("wv", [F, CW], fp16, isOutput=False)
    wo_d = nc.declare_dram_parameter("wo", [CW, F], bf16, isOutput=False)
    bq_d = nc.declare_dram_parameter("bq", [CW, 1], f32, isOutput=False)
    bk_d = nc.declare_dram_parameter("bk", [CW, 1], f32, isOutput=False)
    bv_d = nc.declare_dram_parameter("bv", [1, CW], fp16, isOutput=False)
    msk_d = nc.declare_dram_parameter("msk", [128, 896], f32, isOutput=False)
    # partial outputs are written bf16: the host sums the four partials per
    # batch in fp64, and bf16 rounding of a partial is a ~0.2% effect far
    # inside the tolerance -- while halving the output DMA traffic
    out_d = nc.declare_dram_parameter("out", [S, F], bf16, isOutput=True)

    nsc = S // SC  # 4

    with tile.TileContext(nc) as tc:
        with (
            tc.tile_pool(name="const", bufs=1) as cpool,
            tc.tile_pool(name="xf", bufs=2) as xfpool,
            tc.tile_pool(name="xt", bufs=2) as xtpool,
            tc.tile_pool(name="mskw", bufs=2) as mpool,
            tc.tile_pool(name="work", bufs=2) as wpool,
            tc.tile_pool(name="pbuf", bufs=3) as ppool,
            tc.tile_pool(name="outst", bufs=2) as opool,
            tc.tile_pool(name="ps_gen", bufs=2, space="PSUM") as ps_gen,
            tc.tile_pool(name="ps_pair", bufs=2, space="PSUM") as ps_pair,
            tc.tile_pool(name="ps_av", bufs=2, space="PSUM") as ps_av,
        ):
            # ---- persistent tiles ----
            wq = cpool.tile([128, 8, CW], fp16)
            wk = cpool.tile([128, 8, CW], fp16)
            wv = cpool.tile([128, 8, CW], fp16)
            wo = cpool.tile([128, 2, F], bf16)
            bq = cpool.tile([128, 2, 1], f32)
            bk = cpool.tile([128, 2, 1], f32)
            bv = cpool.tile([1, CW], fp16)
            msk = cpool.tile([128, 896], f32)
            ones1 = cpool.tile([1, 128], f32r)
            ones1h = cpool.tile([1, 128], fp16)
            ones_f = cpool.tile([128, 128], f32)
            qT = cpool.tile([128, 2, S], fp16)
            kT = cpool.tile([128, 2, S], fp16)
            # value path (v1/p/aoT/wo) runs in bf16: halves LDWEIGHTS time
            # (which shares PE issue slots under HAM k=4 duty) and SBUF
            # footprint; the error is linear (no softmax amplification)
            v1 = cpool.tile([128, NZC, HPC, D + 1], bf16)
            aoT = cpool.tile([128, 2, S], bf16)

            # x tiles for all chunks, allocated up-front so DMAs can be
            # issued in priority order (k-interleaved with the weights).
            xfs = [
                xfpool.tile([128, 8, SC], fp16, tag="xf", name=f"xf{i}")
                for i in range(2)
            ]
            xts = [
                xtpool.tile([128, 8, SC], fp16, tag="xt", name=f"xt{i}")
                for i in range(2)
            ]

            # DMAs are split across the two hardware DGE issue queues (sync
            # and scalar) so the startup loads aren't serialized on one
            # queue's ~600ns/instruction issue rate; k-chunks 1-7 are
            # coalesced into a single 3D-AP DMA per tensor.
            def load_x(sc, split=False):
                xf, xt = xfs[sc % 2], xts[sc % 2]
                s0 = sc * SC
                if split:
                    nc.sync.dma_start(
                        out=xf[:, 0, :], in_=xf_d[0:128, s0 : s0 + SC]
                    )
                    nc.scalar.dma_start(
                        out=xt[:, 0, :], in_=xt_d[0:128, s0 : s0 + SC]
                    )
                    nc.sync.dma_start(
                        out=xf[:, 1:8, :],
                        in_=xf_d[128:F, s0 : s0 + SC].rearrange(
                            "(k p) s -> p k s", p=128
                        ),
                    )
                    nc.scalar.dma_start(
                        out=xt[:, 1:8, :],
                        in_=xt_d[128:F, s0 : s0 + SC].rearrange(
                            "(k p) s -> p k s", p=128
                        ),
                    )
                else:
                    nc.sync.dma_start(
                        out=xf[:],
                        in_=xf_d[:, s0 : s0 + SC].rearrange(
                            "(k p) s -> p k s", p=128
                        ),
                    )
                    nc.sync.dma_start(
                        out=xt[:],
                        in_=xt_d[:, s0 : s0 + SC].rearrange(
                            "(k p) s -> p k s", p=128
                        ),
                    )
                return xf, xt

            nc.sync.dma_start(out=wq[:, 0, :], in_=wq_d[0:128, :])
            nc.scalar.dma_start(out=wk[:, 0, :], in_=wk_d[0:128, :])
            xf0, xt0 = load_x(0, split=True)
            nc.sync.dma_start(
                out=wq[:, 1:8, :],
                in_=wq_d[128:F, :].rearrange("(k p) c -> p k c", p=128),
            )
            nc.scalar.dma_start(
                out=wk[:, 1:8, :],
                in_=wk_d[128:F, :].rearrange("(k p) c -> p k c", p=128),
            )
            nc.scalar.dma_start(
                out=bq[:], in_=bq_d[:].rearrange("(m p) c -> p m c", p=128)
            )
            nc.scalar.dma_start(
                out=bk[:], in_=bk_d[:].rearrange("(m p) c -> p m c", p=128)
            )
            nc.scalar.dma_start(
                out=wv[:], in_=wv_d[:].rearrange("(k p) c -> p k c", p=128)
            )
            nc.scalar.dma_start(out=bv[:], in_=bv_d[:])
            nc.sync.dma_start(out=msk[:], in_=msk_d[:])
            nc.sync.dma_start(
                out=wo[:], in_=wo_d[:].rearrange("(m p) c -> p m c", p=128)
            )
            nc.vector.memset(ones_f[:], 1.0)
            nlog2_20 = cpool.tile([1, 1], f32)
            nc.vector.memset(nlog2_20[:], -20.0 * float(np.log(2.0)))
            nc.vector.tensor_copy(ones1[:], ones_f[0:1, :])
            nc.vector.tensor_copy(ones1h[:], ones_f[0:1, :])
            # ones column of v' (col D of each [128, D+1] block)
            nc.vector.tensor_copy(
                v1[:, :, :, D : D + 1],
                ones_f[:, :64].rearrange("p (a b c) -> p a b c", a=NZC, b=HPC),
            )

            # Deferred normalization: the reciprocal starts right after a
            # group's last AV matmul, but the PE broadcast + multiply are
            # emitted only after a few unrelated matmuls have been queued,
            # so the in-order PE queue never stalls on the ACT reciprocal.
            pending = []

            def _fin_b():
                for m_, po_, s0_, pav_, recip_r_ in pending:
                    # pb borrows a slot in the scores pair ring (PSUM is
                    # fully subscribed); the aoT multiply reads both PSUM
                    # operands directly.
                    pbt = ps_pair.tile([128, 2, SC], f32, tag="pair", name="pbt")
                    pb = pbt[0:D, 0, :]
                    nc.tensor.matmul(
                        pb, ones1[:, :D], recip_r_[:], start=True, stop=True
                    )
                    sb = wpool.tile([D, SC], f32, tag="sb")
                    nc.vector.tensor_copy(sb[:], pb)
                    nc.vector.tensor_tensor(
                        out=aoT[po_ : po_ + D, m_, s0_ : s0_ + SC],
                        in0=pav_[0:D, :],
                        in1=sb[:],
                        op=mybir.AluOpType.mult,
                    )
                pending.clear()

            def proj(sc, xf, xt):
                s0 = sc * SC
                for m in range(2):
                    # q and k matmuls interleaved per k-chunk: xf and xt
                    # arrive on parallel DMA queues, so at startup the PE
                    # consumes whichever chunk has landed instead of
                    # stalling on xf alone
                    pq = ps_gen.tile([128, SC], f32, tag="psg")
                    pk = ps_gen.tile([128, SC], f32, tag="psg")
                    for k in range(8):
                        nc.tensor.matmul(
                            pq[:],
                            wq[:, k, m * 128 : (m + 1) * 128],
                            xf[:, k, :],
                            start=(k == 0),
                            stop=(k == 7),
                        )
                        nc.tensor.matmul(
                            pk[:],
                            wk[:, k, m * 128 : (m + 1) * 128],
                            xt[:, k, :],
                            start=(k == 0),
                            stop=(k == 7),
                        )
                    nc.scalar.activation(
                        qT[:, m, s0 : s0 + SC],
                        pq[:],
                        mybir.ActivationFunctionType.Identity,
                        bias=bq[:, m, :],
                    )
                    nc.scalar.activation(
                        kT[:, m, s0 : s0 + SC],
                        pk[:],
                        mybir.ActivationFunctionType.Identity,
                        bias=bk[:, m, :],
                    )
                # flush the previous chunk's last head group here: the
                # 32 q/k matmuls above covered the reciprocal's latency,
                # and the v-proj below covers the broadcast+multiply
                # before outproj(sc-1) reads aoT
                if pending:
                    _fin_b()
                for zz in range(SC // 128):
                    zc = sc * (SC // 128) + zz
                    pv = ps_gen.tile([128, SC], f32, tag="psg")
                    for k in range(8):
                        nc.tensor.matmul(
                            pv[:, :CW],
                            xt[:, k, zz * 128 : (zz + 1) * 128],
                            wv[:, k, :],
                            start=(k == 0),
                            stop=False,
                        )
                    nc.tensor.matmul(
                        pv[:, :CW], ones1h[:, :], bv[:], start=False, stop=True
                    )
                    nc.scalar.copy(
                        v1[:, zc, :, 0:D],
                        pv[:, :CW].rearrange("p (h d) -> p h d", h=HPC),
                    )

            def attn(sc):
                s0 = sc * SC
                nz = (sc + 1) * (SC // 128)
                npr = nz // 2
                for h in range(HPC):
                    m, po = divmod(h, 2)
                    po *= 64
                    pav = ps_av.tile([D + 1, SC], f32, tag="pav")
                    # z-blocks are processed in PAIRS sharing one 2-bank
                    # PSUM tile, so each pair needs only ONE exp activation
                    # ([128,1024]) -- the ACT engine otherwise nearly paces
                    # the PE during the throttled steady state.
                    LEAD = 2  # pairs issued ahead of their AV consumers
                    ptiles = [None] * npr

                    def score_pair(pr):
                        ps2 = ps_pair.tile([128, 2, SC], f32, tag="pair")
                        p2 = ppool.tile([128, 2, SC], bf16, tag="p")
                        z0 = 256 * pr
                        for half in range(2):
                            nc.tensor.matmul(
                                ps2[:, half, :],
                                kT[po : po + D, m, z0 + 128 * half : z0 + 128 * (half + 1)],
                                qT[po : po + D, m, s0 : s0 + SC],
                                start=True,
                                stop=True,
                            )
                        if z0 >= s0:
                            # columns s < z0 are fully masked for both
                            # halves: zero them directly and run the
                            # mask-add + exp only on the live suffix
                            off0 = z0 - s0
                            masked2 = mpool.tile([128, 2, SC], f32, tag="masked")
                            if off0 > 0:
                                nc.vector.memset(p2[:, :, 0:off0], 0.0)
                            for half in range(2):
                                ms = (s0 - z0 - 128 * half) + 384
                                nc.vector.tensor_tensor(
                                    out=masked2[:, half, off0:SC],
                                    in0=ps2[:, half, off0:SC],
                                    in1=msk[:, ms + off0 : ms + SC],
                                    op=mybir.AluOpType.add,
                                )
                            nc.scalar.activation(
                                p2[:, :, off0:SC],
                                masked2[:, :, off0:SC],
                                mybir.ActivationFunctionType.Exp,
                            )
                        else:
                            nc.scalar.activation(
                                p2[:], ps2[:], mybir.ActivationFunctionType.Exp
                            )
                        ptiles[pr] = p2

                    for pr in range(min(LEAD, npr)):
                        score_pair(pr)
                    for pr in range(npr):
                        if pr + LEAD < npr:
                            score_pair(pr + LEAD)
                        for half in range(2):
                            zc = 2 * pr + half
                            nc.tensor.matmul(
                                pav[:],
                                v1[:, zc, h, :],
                                ptiles[pr][:, half, :],
                                start=(zc == 0),
                                stop=(zc == nz - 1),
                            )
                        ptiles[pr] = None
                        if pr == min(2, npr - 1) and pending:
                            _fin_b()
                    # 1/x as Exp(-Ln(x)) on the scalar engine: two fast ACT
                    # passes (~1.2us) instead of the 3.4us single-lane DVE
                    # reciprocal that stalled the PE at group boundaries.
                    # ln/exp/identity/copy all live in the same ACT function
                    # set (natural_log_exp_and_others) -> no table reloads.
                    # No zero-denominator clamp: a zero only occurs for the
                    # fully-masked row 0, which the host overwrites.
                    # The HW ln table is only valid for x in [2^-64, 2^64];
                    # denominators span [2^-30, 2^74], so shift by 2^-20
                    # inside the Ln and compensate in the Exp's bias:
                    # recip = exp(-(ln(den*2^-20)) - 20*ln2) = 1/den.
                    lnd = wpool.tile([1, SC], f32, tag="lnd")
                    nc.scalar.activation(
                        lnd[:],
                        pav[D : D + 1, :],
                        mybir.ActivationFunctionType.Ln,
                        scale=2.0**-20,
                    )
                    recip_r = wpool.tile([1, SC], f32r, tag="recip_r")
                    with nc.allow_low_precision(
                        reason="softmax denom reciprocal: f32r rounding is a "
                        "0.05% common-mode factor, well within tolerance"
                    ):
                        nc.scalar.activation(
                            recip_r[:],
                            lnd[:],
                            mybir.ActivationFunctionType.Exp,
                            scale=-1.0,
                            bias=nlog2_20[:],
                        )
                    pending.append((m, po, s0, pav, recip_r))

            def outproj(sc):
                # out rows s0..s0+SC, consuming aoT for all 4 heads at sc;
                # staging copies run on the (idle) DVE, not the hot ACT
                for so in range(sc * (SC // 128), (sc + 1) * (SC // 128)):
                    s0 = so * 128
                    ost = opool.tile([128, F], bf16, tag="ost")
                    for fo in range(2):
                        po_ = ps_gen.tile([128, SC], f32, tag="psg")
                        for m in range(2):
                            nc.tensor.matmul(
                                po_[:],
                                aoT[:, m, s0 : s0 + 128],
                                wo[:, m, fo * SC : (fo + 1) * SC],
                                start=(m == 0),
                                stop=(m == 1),
                            )
                        nc.vector.tensor_copy(
                            ost[:, fo * SC : (fo + 1) * SC], po_[:]
                        )
                    nc.sync.dma_start(out=out_d[s0 : s0 + 128, :], in_=ost[:])

            # ---- fused pipeline over s-chunks ----
            xf, xt = xf0, xt0
            for sc in range(nsc):
                if sc + 1 < nsc:
                    nxf, nxt = load_x(sc + 1)
                proj(sc, xf, xt)
                if sc > 0:
                    # previous chunk's out rows: its last head group's
                    # normalize was flushed at the top of proj(sc), with
                    # proj's remaining matmuls covering the latency
                    outproj(sc - 1)
                attn(sc)
                if sc + 1 < nsc:
                    xf, xt = nxf, nxt
            _fin_b()
            outproj(nsc - 1)

    _split_excess_waits(nc)
    return nc


_CACHE = {}


def _get_nc():
    if "nc" not in _CACHE:
        _CACHE["nc"] = _build()
    return _CACHE["nc"]


def _ensure_ntff_hook():
    """The agent image's antenv lacks axon_hooks, so run_bass_kernel_spmd's
    trace path can't import it. Synthesize the module and install the
    ctypes NTFF hook from trn_agent_boot (same thing boot() would do)."""
    import sys
    import types

    if "antenv.axon_hooks" not in sys.modules:
        mod = types.ModuleType("antenv.axon_hooks")
        holder = [None]
        mod.set_axon_ntff_profile_hook = lambda h: holder.__setitem__(0, h)
        mod.get_axon_ntff_profile_hook = lambda: holder[0]
        sys.modules["antenv.axon_hooks"] = mod
        import antenv

        antenv.axon_hooks = mod
    import antenv.axon_hooks as ah

    if ah.get_axon_ntff_profile_hook() is None:
        try:
            from trn_agent_boot.trn_boot import _ntff_profile_via_ctypes

            ah.set_axon_ntff_profile_hook(
                _ntff_profile_via_ctypes("/opt/axon/libaxon_pjrt.so")
            )
        except Exception:
            pass


def _host_mask():
    i = np.arange(128)[:, None]
    m = np.arange(896)[None, :]
    return np.where(i >= (m - 384), -np.float32(MASK_VAL), np.float32(0.0)).astype(
        np.float32
    )


def kernel(attend_from, attend_to, w_q, b_q, w_kv, b_kv, w_out, b_out, _trace=False):
    attend_from = np.asarray(attend_from, dtype=np.float32)
    attend_to = np.asarray(attend_to, dtype=np.float32)
    w_q = np.asarray(w_q, dtype=np.float32)
    b_q = np.asarray(b_q, dtype=np.float32)
    w_kv = np.asarray(w_kv, dtype=np.float32)
    b_kv = np.asarray(b_kv, dtype=np.float32)
    w_out = np.asarray(w_out, dtype=np.float32)
    b_out = np.asarray(b_out, dtype=np.float32)

    msk = _host_mask()
    xT = [np.ascontiguousarray(attend_from[b].T).astype(np.float16) for b in range(B)]
    xTt = [np.ascontiguousarray(attend_to[b].T).astype(np.float16) for b in range(B)]

    in_maps = []
    for c in range(NCORES):
        b, hg = divmod(c, HG)
        cols = slice(hg * CW, (hg + 1) * CW)
        in_maps.append(
            {
                "xf": xT[b],
                "xt": xTt[b],
                "wq": np.ascontiguousarray(w_q[:, cols]).astype(np.float16),
                "wk": np.ascontiguousarray(w_kv[:, cols]).astype(np.float16),
                "wv": np.ascontiguousarray(w_kv[:, F:][:, cols]).astype(np.float16),
                "wo": np.ascontiguousarray(w_out[cols, :]).astype(ml_dtypes.bfloat16),
                "bq": np.ascontiguousarray(b_q[cols].reshape(CW, 1)),
                "bk": np.ascontiguousarray(b_kv[cols].reshape(CW, 1)),
                "bv": np.ascontiguousarray(b_kv[F:][cols].reshape(1, CW)).astype(np.float16),
                "msk": msk,
                "out": np.zeros((S, F), ml_dtypes.bfloat16),
            }
        )

    nc = _get_nc()
    if _trace:
        _ensure_ntff_hook()
    res = run_bass_kernel_spmd(nc, in_maps, list(range(NCORES)), trace=_trace)

    out = np.zeros((B, S, F), np.float64)
    for c in range(NCORES):
        b = c // HG
        out[b] += res.results[c]["out"].astype(np.float64)
    out += b_out.astype(np.float64)[None, None, :]

    # Row 0 of the reference is fully masked -> softmax is exactly uniform
    # over all Z positions; compute it directly on the host.
    w_v = w_kv[:, F:].astype(np.float64)
    for b in range(B):
        val_mean = attend_to[b].astype(np.float64).mean(axis=0) @ w_v + b_kv[
            F:
        ].astype(np.float64)
        out[b, 0, :] = val_mean @ w_out.astype(np.float64) + b_out.astype(np.float64)

    kernel._last_result = res
    return out.astype(np.float32)


# revision 41
# speedup vs baseline: 1.0831x; 1.0831x over previous
"""Causal cross-attention kernel for 8 TRN2 NeuronCores.

Sharding: data-parallel over batch (B=2) x tensor-parallel over head
groups (16 heads -> 4 groups of 4). Core c handles batch c//4, heads
[4*(c%4), 4*(c%4)+4). Each core computes its partial output projection
(w_out rows for its heads); the host sums the 4 partials per batch
(the "all-reduce"), adds b_out, and fixes the fully-masked row 0.

The kernel is PE-issue-limited under the HAM power throttle (sustained
PE duty is capped at ~50% after a ~65us full-rate grace window), so the
structure maximizes overlap: a single fused pipeline over s-chunks
  proj(sc) -> attention(sc, all heads) -> out-proj(sc)
with DMAs issued k-interleaved so the first matmul starts ~5us in, and
score/AV issue interleaved so the PE rides through the mask->exp
round-trip latency.

Device dataflow per core (all matmuls in natural layouts, no device
transposes; activations host-transposed once):
  qT[f',s] = sum_f wq[f,f'] xT_from[f,s]     (lhsT=wq tile, rhs=xT tile)
  kT[f',s] = sum_f wk[f,f'] xT_to[f,s]
  v[z,f']  = sum_f xT_to[f,z] wv[f,f']       (lhsT=xT tile, rhs=wv)
  scoresT[z,s] = sum_d kT[d,z] qT[d,s]       (per head, K=64)
  P = exp(scoresT + causal_mask)             (no max-subtraction; logits
                                              are bounded ~|50| so exp is
                                              safe in f32, masked -> 0)
  out'T[d|1,s] = sum_z v'[z,d|1] P[z,s]      (v' has a ones column ->
                                              row 64 = softmax denom)
  attn_outT = out'T[0:64] * (1/out'T[64])    (PE-broadcast of recip row)
  out[s,fo] = sum_hd attn_outT[hd,s] wo[hd,fo]
"""

import numpy as np
import ml_dtypes
import concourse.bass as bass
import concourse.mybir as mybir
import concourse.tile as tile
from concourse.bass_utils import run_bass_kernel_spmd

B, S, F, H = 2, 2048, 1024, 16
NCORES = 8
HG = 4          # head groups (tensor-parallel degree per batch)
HPC = H // HG   # heads per core = 4
D = F // H      # head dim = 64
CW = HPC * D    # per-core projection width = 256
MASK_VAL = 1.0e12
SC = 512        # s-chunk for projections / scores
NZC = S // 128  # 16 z-chunks

f32 = mybir.dt.float32
f32r = mybir.dt.float32r
bf16 = mybir.dt.bfloat16
fp16 = mybir.dt.float16

# Walrus encodes at most 1 sync wait on most TRN2 instructions; Tile can
# attach several. Redistribute excess waits onto preceding same-engine NOPs.


def _split_excess_waits(nc):
    for fn in nc.m.functions:
        for bb in fn.blocks:
            insts = list(bb.instructions)
            out = []
            changed = False
            for inst in insts:
                si = inst.sync_info
                waits = list(si.on_wait) if si is not None else []
                if len(waits) > 1:
                    changed = True
                    inst.sync_info = mybir.SyncInfo(
                        on_update=list(si.on_update), on_wait=waits[-1:]
                    )
                    for idx, w in enumerate(waits[:-1]):
                        nop = mybir.InstNoOp(name=f"{inst.name}-wsplit{idx}")
                        nop.engine = inst.engine
                        nop.sync_info = mybir.SyncInfo(on_update=[], on_wait=[w])
                        out.append(nop)
                out.append(inst)
            if changed:
                bb.instructions = out


def _round_f32r(x):
    u = np.ascontiguousarray(x, dtype=np.float32).view(np.uint32)
    u = ((u.astype(np.uint64) + 0x1000) & 0xFFFFE000).astype(np.uint32)
    return u.view(np.float32)


def _build():
    nc = bass.Bass()
    xf_d = nc.declare_dram_parameter("xf", [F, S], fp16, isOutput=False)
    xt_d = nc.declare_dram_parameter("xt", [F, S], fp16, isOutput=False)
    wq_d = nc.declare_dram_parameter("wq", [F, CW], fp16, isOutput=False)
    wk_d = nc.declare_dram_parameter("wk", [F, CW], fp16, isOutput=False)
    wv_d = nc.declare_dram_parameter("wv", [F, CW], fp16, isOutput=False)
    wo_d = nc.declare_dram_parameter("wo", [CW, F], bf16, isOutput=False)
    bq_d = nc.declare_dram_parameter("bq", [CW, 1], f32, isOutput=False)
    bk_d = nc.declare_dram_parameter("bk", [CW, 1], f32, isOutput=False)
    bv_d = nc.declare_dram_parameter("bv", [1, CW], fp16, isOutput=False)
    msk_d = nc.declare_dram_parameter("msk", [128, 896], f32, isOutput=False)
    # partial outputs are written bf16: the host sums the four partials per
    # batch in fp64, and bf16 rounding of a partial is a ~0.2% effect far
    # inside the tolerance -- while halving the output DMA traffic
    out_d = nc.declare_dram_parameter("out", [S, F], bf16, isOutput=True)

    nsc = S // SC  # 4

    with tile.TileContext(nc) as tc:
        with (
            tc.tile_pool(name="const", bufs=1) as cpool,
            tc.tile_pool(name="xf", bufs=2) as xfpool,
            tc.tile_pool(name="xt", bufs=2) as xtpool,
            tc.tile_pool(name="mskw", bufs=2) as mpool,
            tc.tile_pool(name="work", bufs=2) as wpool,
            tc.tile_pool(name="pbuf", bufs=3) as ppool,
            tc.tile_pool(name="outst", bufs=2) as opool,
            tc.tile_pool(name="ps_gen", bufs=2, space="PSUM") as ps_gen,
            tc.tile_pool(name="ps_pair", bufs=2, space="PSUM") as ps_pair,
            tc.tile_pool(name="ps_av", bufs=2, space="PSUM") as ps_av,
        ):
            # ---- persistent tiles ----
            wq = cpool.tile([128, 8, CW], fp16)
            wk = cpool.tile([128, 8, CW], fp16)
            wv = cpool.tile([128, 8, CW], fp16)
            wo = cpool.tile([128, 2, F], bf16)
            bq = cpool.tile([128, 2, 1], f32)
            bk = cpool.tile([128, 2, 1], f32)
            bv = cpool.tile([1, CW], fp16)
            msk = cpool.tile([128, 896], f32)
            ones1 = cpool.tile([1, 128], f32r)
            ones1h = cpool.tile([1, 128], fp16)
            ones_f = cpool.tile([128, 128], f32)
            qT = cpool.tile([128, 2, S], fp16)
            kT = cpool.tile([128, 2, S], fp16)
            # value path (v1/p/aoT/wo) runs in bf16: halves LDWEIGHTS time
            # (which shares PE issue slots under HAM k=4 duty) and SBUF
            # footprint; the error is linear (no softmax amplification)
            v1 = cpool.tile([128, NZC, HPC, D + 1], bf16)
            aoT = cpool.tile([128, 2, S], bf16)

            # x tiles for all chunks, allocated up-front so DMAs can be
            # issued in priority order (k-interleaved with the weights).
            xfs = [
                xfpool.tile([128, 8, SC], fp16, tag="xf", name=f"xf{i}")
                for i in range(2)
            ]
            xts = [
                xtpool.tile([128, 8, SC], fp16, tag="xt", name=f"xt{i}")
                for i in range(2)
            ]

            # DMAs are split across the two hardware DGE issue queues (sync
            # and scalar) so the startup loads aren't serialized on one
            # queue's ~600ns/instruction issue rate; k-chunks 1-7 are
            # coalesced into a single 3D-AP DMA per tensor.
            def load_x(sc, split=False):
                xf, xt = xfs[sc % 2], xts[sc % 2]
                s0 = sc * SC
                if split:
                    nc.sync.dma_start(
                        out=xf[:, 0, :], in_=xf_d[0:128, s0 : s0 + SC]
                    )
                    nc.scalar.dma_start(
                        out=xt[:, 0, :], in_=xt_d[0:128, s0 : s0 + SC]
                    )
                    nc.sync.dma_start(
                        out=xf[:, 1:8, :],
                        in_=xf_d[128:F, s0 : s0 + SC].rearrange(
                            "(k p) s -> p k s", p=128
                        ),
                    )
                    nc.scalar.dma_start(
                        out=xt[:, 1:8, :],
                        in_=xt_d[128:F, s0 : s0 + SC].rearrange(
                            "(k p) s -> p k s", p=128
                        ),
                    )
                else:
                    nc.sync.dma_start(
                        out=xf[:],
                        in_=xf_d[:, s0 : s0 + SC].rearrange(
                            "(k p) s -> p k s", p=128
                        ),
                    )
                    nc.sync.dma_start(
                        out=xt[:],
                        in_=xt_d[:, s0 : s0 + SC].rearrange(
                            "(k p) s -> p k s", p=128
                        ),
                    )
                return xf, xt

            nc.sync.dma_start(out=wq[:, 0, :], in_=wq_d[0:128, :])
            nc.scalar.dma_start(out=wk[:, 0, :], in_=wk_d[0:128, :])
            xf0, xt0 = load_x(0, split=True)
            nc.sync.dma_start(
                out=wq[:, 1:8, :],
                in_=wq_d[128:F, :].rearrange("(k p) c -> p k c", p=128),
            )
            nc.scalar.dma_start(
                out=wk[:, 1:8, :],
                in_=wk_d[128:F, :].rearrange("(k p) c -> p k c", p=128),
            )
            nc.scalar.dma_start(
                out=bq[:], in_=bq_d[:].rearrange("(m p) c -> p m c", p=128)
            )
            nc.scalar.dma_start(
                out=bk[:], in_=bk_d[:].rearrange("(m p) c -> p m c", p=128)
            )
            nc.scalar.dma_start(
                out=wv[:], in_=wv_d[:].rearrange("(k p) c -> p k c", p=128)
            )
            nc.scalar.dma_start(out=bv[:], in_=bv_d[:])
            nc.sync.dma_start(out=msk[:], in_=msk_d[:])
            nc.sync.dma_start(
                out=wo[:], in_=wo_d[:].rearrange("(m p) c -> p m c", p=128)
            )
            nc.vector.memset(ones_f[:], 1.0)
            nlog2_20 = cpool.tile([1, 1], f32)
            nc.vector.memset(nlog2_20[:], -20.0 * float(np.log(2.0)))
            nc.vector.tensor_copy(ones1[:], ones_f[0:1, :])
            nc.vector.tensor_copy(ones1h[:], ones_f[0:1, :])
            # ones column of v' (col D of each [128, D+1] block)
            nc.vector.tensor_copy(
                v1[:, :, :, D : D + 1],
                ones_f[:, :64].rearrange("p (a b c) -> p a b c", a=NZC, b=HPC),
            )

            # Deferred normalization: the reciprocal starts right after a
            # group's last AV matmul, but the PE broadcast + multiply are
            # emitted only after a few unrelated matmuls have been queued,
            # so the in-order PE queue never stalls on the ACT reciprocal.
            pending = []

            def _fin_b():
                for m_, po_, s0_, pav_, recip_r_ in pending:
                    # pb borrows a slot in the scores pair ring (PSUM is
                    # fully subscribed); the aoT multiply reads both PSUM
                    # operands directly.
                    pbt = ps_pair.tile([128, 2, SC], f32, tag="pair", name="pbt")
                    pb = pbt[0:D, 0, :]
                    nc.tensor.matmul(
                        pb, ones1[:, :D], recip_r_[:], start=True, stop=True
                    )
                    sb = wpool.tile([D, SC], f32, tag="sb")
                    nc.vector.tensor_copy(sb[:], pb)
                    nc.vector.tensor_tensor(
                        out=aoT[po_ : po_ + D, m_, s0_ : s0_ + SC],
                        in0=pav_[0:D, :],
                        in1=sb[:],
                        op=mybir.AluOpType.mult,
                    )
                pending.clear()

            def proj(sc, xf, xt):
                s0 = sc * SC
                for m in range(2):
                    # q and k matmuls interleaved per k-chunk: xf and xt
                    # arrive on parallel DMA queues, so at startup the PE
                    # consumes whichever chunk has landed instead of
                    # stalling on xf alone
                    pq = ps_gen.tile([128, SC], f32, tag="psg")
                    pk = ps_gen.tile([128, SC], f32, tag="psg")
                    for k in range(8):
                        nc.tensor.matmul(
                            pq[:],
                            wq[:, k, m * 128 : (m + 1) * 128],
                            xf[:, k, :],
                            start=(k == 0),
                            stop=(k == 7),
                        )
                        nc.tensor.matmul(
                            pk[:],
                            wk[:, k, m * 128 : (m + 1) * 128],
                            xt[:, k, :],
                            start=(k == 0),
                            stop=(k == 7),
                        )
                    nc.scalar.activation(
                        qT[:, m, s0 : s0 + SC],
                        pq[:],
                        mybir.ActivationFunctionType.Identity,
                        bias=bq[:, m, :],
                    )
                    nc.scalar.activation(
                        kT[:, m, s0 : s0 + SC],
                        pk[:],
                        mybir.ActivationFunctionType.Identity,
                        bias=bk[:, m, :],
                    )
                # flush the previous chunk's last head group here: the
                # 32 q/k matmuls above covered the reciprocal's latency,
                # and the v-proj below covers the broadcast+multiply
                # before outproj(sc-1) reads aoT
                if pending:
                    _fin_b()
                for zz in range(SC // 128):
                    zc = sc * (SC // 128) + zz
                    pv = ps_gen.tile([128, SC], f32, tag="psg")
                    for k in range(8):
                        nc.tensor.matmul(
                            pv[:, :CW],
                            xt[:, k, zz * 128 : (zz + 1) * 128],
                            wv[:, k, :],
                            start=(k == 0),
                            stop=False,
                        )
                    nc.tensor.matmul(
                        pv[:, :CW], ones1h[:, :], bv[:], start=False, stop=True
                    )
                    nc.vector.tensor_copy(
                        v1[:, zc, :, 0:D],
                        pv[:, :CW].rearrange("p (h d) -> p h d", h=HPC),
                    )

            def attn(sc):
                s0 = sc * SC
                nz = (sc + 1) * (SC // 128)
                npr = nz // 2
                for h in range(HPC):
                    m, po = divmod(h, 2)
                    po *= 64
                    pav = ps_av.tile([D + 1, SC], f32, tag="pav")
                    # z-blocks are processed in PAIRS sharing one 2-bank
                    # PSUM tile, so each pair needs only ONE exp activation
                    # ([128,1024]) -- the ACT engine otherwise nearly paces
                    # the PE during the throttled steady state.
                    LEAD = 2  # pairs issued ahead of their AV consumers
                    ptiles = [None] * npr

                    def score_pair(pr):
                        ps2 = ps_pair.tile([128, 2, SC], f32, tag="pair")
                        p2 = ppool.tile([128, 2, SC], bf16, tag="p")
                        z0 = 256 * pr
                        for half in range(2):
                            nc.tensor.matmul(
                                ps2[:, half, :],
                                kT[po : po + D, m, z0 + 128 * half : z0 + 128 * (half + 1)],
                                qT[po : po + D, m, s0 : s0 + SC],
                                start=True,
                                stop=True,
                            )
                        if z0 >= s0:
                            # columns s < z0 are fully masked for both
                            # halves: zero them directly and run the
                            # mask-add + exp only on the live suffix
                            off0 = z0 - s0
                            masked2 = mpool.tile([128, 2, SC], f32, tag="masked")
                            if off0 > 0:
                                nc.vector.memset(p2[:, :, 0:off0], 0.0)
                            for half in range(2):
                                ms = (s0 - z0 - 128 * half) + 384
                                nc.vector.tensor_tensor(
                                    out=masked2[:, half, off0:SC],
                                    in0=ps2[:, half, off0:SC],
                                    in1=msk[:, ms + off0 : ms + SC],
                                    op=mybir.AluOpType.add,
                                )
                            nc.scalar.activation(
                                p2[:, :, off0:SC],
                                masked2[:, :, off0:SC],
                                mybir.ActivationFunctionType.Exp,
                            )
                        else:
                            nc.scalar.activation(
                                p2[:], ps2[:], mybir.ActivationFunctionType.Exp
                            )
                        ptiles[pr] = p2

                    for pr in range(min(LEAD, npr)):
                        score_pair(pr)
                    for pr in range(npr):
                        if pr + LEAD < npr:
                            score_pair(pr + LEAD)
                        for half in range(2):
                            zc = 2 * pr + half
                            nc.tensor.matmul(
                                pav[:],
                                v1[:, zc, h, :],
                                ptiles[pr][:, half, :],
                                start=(zc == 0),
                                stop=(zc == nz - 1),
                            )
                        ptiles[pr] = None
                        if pr == min(2, npr - 1) and pending:
                            _fin_b()
                    # 1/x as Exp(-Ln(x)) on the scalar engine: two fast ACT
                    # passes (~1.2us) instead of the 3.4us single-lane DVE
                    # reciprocal that stalled the PE at group boundaries.
                    # ln/exp/identity/copy all live in the same ACT function
                    # set (natural_log_exp_and_others) -> no table reloads.
                    # No zero-denominator clamp: a zero only occurs for the
                    # fully-masked row 0, which the host overwrites.
                    # The HW ln table is only valid for x in [2^-64, 2^64];
                    # denominators span [2^-30, 2^74], so shift by 2^-20
                    # inside the Ln and compensate in the Exp's bias:
                    # recip = exp(-(ln(den*2^-20)) - 20*ln2) = 1/den.
                    lnd = wpool.tile([1, SC], f32, tag="lnd")
                    nc.scalar.activation(
                        lnd[:],
                        pav[D : D + 1, :],
                        mybir.ActivationFunctionType.Ln,
                        scale=2.0**-20,
                    )
                    recip_r = wpool.tile([1, SC], f32r, tag="recip_r")
                    with nc.allow_low_precision(
                        reason="softmax denom reciprocal: f32r rounding is a "
                        "0.05% common-mode factor, well within tolerance"
                    ):
                        nc.scalar.activation(
                            recip_r[:],
                            lnd[:],
                            mybir.ActivationFunctionType.Exp,
                            scale=-1.0,
                            bias=nlog2_20[:],
                        )
                    pending.append((m, po, s0, pav, recip_r))

            def outproj(sc):
                # out rows s0..s0+SC, consuming aoT for all 4 heads at sc;
                # staging copies run on the (idle) DVE, not the hot ACT
                for so in range(sc * (SC // 128), (sc + 1) * (SC // 128)):
                    s0 = so * 128
                    ost = opool.tile([128, F], bf16, tag="ost")
                    for fo in range(2):
                        po_ = ps_gen.tile([128, SC], f32, tag="psg")
                        for m in range(2):
                            nc.tensor.matmul(
                                po_[:],
                                aoT[:, m, s0 : s0 + 128],
                                wo[:, m, fo * SC : (fo + 1) * SC],
                                start=(m == 0),
                                stop=(m == 1),
                            )
                        nc.vector.tensor_copy(
                            ost[:, fo * SC : (fo + 1) * SC], po_[:]
                        )
                    nc.sync.dma_start(out=out_d[s0 : s0 + 128, :], in_=ost[:])

            # ---- fused pipeline over s-chunks ----
            xf, xt = xf0, xt0
            for sc in range(nsc):
                if sc + 1 < nsc:
                    nxf, nxt = load_x(sc + 1)
                proj(sc, xf, xt)
                if sc > 0:
                    # previous chunk's out rows: its last head group's
                    # normalize was flushed at the top of proj(sc), with
                    # proj's remaining matmuls covering the latency
                    outproj(sc - 1)
                attn(sc)
                if sc + 1 < nsc:
                    xf, xt = nxf, nxt
            _fin_b()
            outproj(nsc - 1)

    _split_excess_waits(nc)
    return nc


_CACHE = {}


def _get_nc():
    if "nc" not in _CACHE:
        _CACHE["nc"] = _build()
    return _CACHE["nc"]


def _ensure_ntff_hook():
    """The agent image's antenv lacks axon_hooks, so run_bass_kernel_spmd's
    trace path can't import it. Synthesize the module and install the
    ctypes NTFF hook from trn_agent_boot (same thing boot() would do)."""
    import sys
    import types

    if "antenv.axon_hooks" not in sys.modules:
        mod = types.ModuleType("antenv.axon_hooks")
        holder = [None]
        mod.set_axon_ntff_profile_hook = lambda h: holder.__setitem__(0, h)
        mod.get_axon_ntff_profile_hook = lambda: holder[0]
        sys.modules["antenv.axon_hooks"] = mod
        import antenv

        antenv.axon_hooks = mod
    import antenv.axon_hooks as ah

    if ah.get_axon_ntff_profile_hook() is None:
        try:
            from trn_agent_boot.trn_boot import _ntff_profile_via_ctypes

            ah.set_axon_ntff_profile_hook(
                _ntff_profile_via_ctypes("/opt/axon/libaxon_pjrt.so")
            )
        except Exception:
            pass


def _host_mask():
    i = np.arange(128)[:, None]
    m = np.arange(896)[None, :]
    return np.where(i >= (m - 384), -np.float32(MASK_VAL), np.float32(0.0)).astype(
        np.float32
    )


def kernel(attend_from, attend_to, w_q, b_q, w_kv, b_kv, w_out, b_out, _trace=False):
    attend_from = np.asarray(attend_from, dtype=np.float32)
    attend_to = np.asarray(attend_to, dtype=np.float32)
    w_q = np.asarray(w_q, dtype=np.float32)
    b_q = np.asarray(b_q, dtype=np.float32)
    w_kv = np.asarray(w_kv, dtype=np.float32)
    b_kv = np.asarray(b_kv, dtype=np.float32)
    w_out = np.asarray(w_out, dtype=np.float32)
    b_out = np.asarray(b_out, dtype=np.float32)

    msk = _host_mask()
    xT = [np.ascontiguousarray(attend_from[b].T).astype(np.float16) for b in range(B)]
    xTt = [np.ascontiguousarray(attend_to[b].T).astype(np.float16) for b in range(B)]

    in_maps = []
    for c in range(NCORES):
        b, hg = divmod(c, HG)
        cols = slice(hg * CW, (hg + 1) * CW)
        in_maps.append(
            {
                "xf": xT[b],
                "xt": xTt[b],
                "wq": np.ascontiguousarray(w_q[:, cols]).astype(np.float16),
                "wk": np.ascontiguousarray(w_kv[:, cols]).astype(np.float16),
                "wv": np.ascontiguousarray(w_kv[:, F:][:, cols]).astype(np.float16),
                "wo": np.ascontiguousarray(w_out[cols, :]).astype(ml_dtypes.bfloat16),
                "bq": np.ascontiguousarray(b_q[cols].reshape(CW, 1)),
                "bk": np.ascontiguousarray(b_kv[cols].reshape(CW, 1)),
                "bv": np.ascontiguousarray(b_kv[F:][cols].reshape(1, CW)).astype(np.float16),
                "msk": msk,
                "out": np.zeros((S, F), ml_dtypes.bfloat16),
            }
        )

    nc = _get_nc()
    if _trace:
        _ensure_ntff_hook()
    res = run_bass_kernel_spmd(nc, in_maps, list(range(NCORES)), trace=_trace)

    out = np.zeros((B, S, F), np.float64)
    for c in range(NCORES):
        b = c // HG
        out[b] += res.results[c]["out"].astype(np.float64)
    out += b_out.astype(np.float64)[None, None, :]

    # Row 0 of the reference is fully masked -> softmax is exactly uniform
    # over all Z positions; compute it directly on the host.
    w_v = w_kv[:, F:].astype(np.float64)
    for b in range(B):
        val_mean = attend_to[b].astype(np.float64).mean(axis=0) @ w_v + b_kv[
            F:
        ].astype(np.float64)
        out[b, 0, :] = val_mean @ w_out.astype(np.float64) + b_out.astype(np.float64)

    kernel._last_result = res
    return out.astype(np.float32)
